# revision 1
# baseline (speedup 1.0000x reference)
"""Trainium2 Bass kernel for nn_LocalDenseCrossReadout.

Strategy:
- Data-parallel over batch: 8 batches -> 8 NeuronCores, one batch per core.
- Host-side (numpy, ~0.1% of FLOPs): FiLM conditioning (ctx -> gamma/beta),
  folding of LayerNorm affine + FiLM + score scale into the projection
  weights, and band-slicing of the additive mask.
- Device kernel per core: LayerNorm stats+apply for q [1024,512] and
  source [4096,512], transposed projections (f32r matmuls), banded local
  attention (768-wide aligned kv window per 128-row q tile), low-rank gate
  bias, softmax, attn@V and output projection.
"""

import sys

sys.path.insert(0, "/opt/trn_rl_repo")

import numpy as np

import concourse.bass as bass
import concourse.tile as tile
from concourse import bacc
from concourse import mybir
from concourse.bass_utils import run_bass_kernel_spmd
from concourse.masks import make_identity

DIM, QS, QT, KS, KT, WIN, B, RANK = 512, 64, 16, 256, 16, 4, 8, 32
Q = QS * QT  # 1024
K = KS * KT  # 4096
WINW = 768  # aligned kv window per 128-row q tile
NQT = Q // 128  # 8 q tiles
F32 = mybir.dt.float32
F32R = mybir.dt.float32r
FT = mybir.ActivationFunctionType
ALU = mybir.AluOpType
AX = mybir.AxisListType

# kv window start (aligned to 128) per q tile; phase split of the kv axis
WSTARTS = [0, 384, 896, 1408, 1920, 2432, 2944, 3328]
PHASES = [  # (kv_start, n_kv_tiles, q_tiles)
    (0, 17, range(0, 4)),
    (1920, 17, range(4, 8)),
]
KVW = 17 * 128  # 2176 kv columns held on-chip per phase


def r32(ap):
    return ap.bitcast(F32R)


def build_bass(debug=False, stage=5):
    nc = bacc.Bacc("TRN2", target_bir_lowering=False)
    q = nc.dram_tensor("q", [Q, DIM], F32, kind="ExternalInput")
    s = nc.dram_tensor("s", [K, DIM], F32, kind="ExternalInput")
    wq = nc.dram_tensor("wq", [DIM, DIM], F32R, kind="ExternalInput")
    wk = nc.dram_tensor("wk", [DIM, DIM], F32R, kind="ExternalInput")
    wv = nc.dram_tensor("wv", [DIM, DIM], F32R, kind="ExternalInput")
    wo = nc.dram_tensor("wo", [DIM, DIM], F32R, kind="ExternalInput")
    wgq = nc.dram_tensor("wgq", [DIM, RANK], F32R, kind="ExternalInput")
    wgk = nc.dram_tensor("wgk", [DIM, RANK], F32R, kind="ExternalInput")
    rqt = nc.dram_tensor("rqt", [128, 4], F32, kind="ExternalInput")
    rkt = nc.dram_tensor("rkt", [128, 4], F32, kind="ExternalInput")
    rv = nc.dram_tensor("rv", [1, DIM], F32R, kind="ExternalInput")
    bo = nc.dram_tensor("bo", [1, DIM], F32R, kind="ExternalInput")
    bmask = nc.dram_tensor("bmask", [NQT, 128, WINW], F32, kind="ExternalInput")
    out = nc.dram_tensor("out", [Q, DIM], F32, kind="ExternalOutput")
    if debug:
        d_qpT = nc.dram_tensor("d_qpT", [128, 4, Q], F32, kind="ExternalOutput")
        d_gq = nc.dram_tensor("d_gq", [32, Q], F32, kind="ExternalOutput")
        d_kT = nc.dram_tensor("d_kT", [128, 4, KVW], F32, kind="ExternalOutput")
        d_vb = nc.dram_tensor("d_vb", [128, 17, DIM], F32, kind="ExternalOutput")
        d_gk = nc.dram_tensor("d_gk", [32, KVW], F32, kind="ExternalOutput")
        d_S = nc.dram_tensor("d_S", [128, WINW], F32, kind="ExternalOutput")
        d_P = nc.dram_tensor("d_P", [128, WINW], F32, kind="ExternalOutput")
        d_oa = nc.dram_tensor("d_oa", [128, DIM], F32, kind="ExternalOutput")

    with tile.TileContext(nc) as tc:
        with (
            tc.tile_pool(name="consts", bufs=1) as consts,
            tc.tile_pool(name="wts", bufs=1) as wts,
            tc.tile_pool(name="kv", bufs=1) as kvpool,
            tc.tile_pool(name="xin", bufs=3) as xin,
            tc.tile_pool(name="stats", bufs=4) as stats,
            tc.tile_pool(name="xt", bufs=2) as xtp,
            tc.tile_pool(name="attn", bufs=2) as attn,
            tc.tile_pool(name="outp", bufs=2) as outp,
            tc.tile_pool(name="ps_s", bufs=3, space="PSUM") as ps_s,
            tc.tile_pool(name="ps_b", bufs=2, space="PSUM") as ps_b,
        ):
            # ---------------- constants ----------------
            ident = consts.tile([128, 128], F32)
            make_identity(nc, ident)
            eps = consts.tile([128, 1], F32)
            nc.vector.memset(eps, 1e-5)
            ones1 = consts.tile([1, 128], F32R)
            nc.vector.memset(ones1.bitcast(F32), 1.0)
            zero_c = consts.tile([128, 1], F32)
            nc.vector.memset(zero_c, 0.0)
            eps6 = consts.tile([128, 1], F32)
            nc.vector.memset(eps6, 1e-6)
            rqt_sb = consts.tile([128, 4], F32)
            nc.sync.dma_start(out=rqt_sb, in_=rqt[:, :])
            rkt_sb = consts.tile([128, 4], F32)
            nc.sync.dma_start(out=rkt_sb, in_=rkt[:, :])
            rv_sb = consts.tile([1, DIM], F32R)
            nc.sync.dma_start(out=rv_sb, in_=rv[:, :])
            bo_sb = consts.tile([1, DIM], F32R)
            nc.sync.dma_start(out=bo_sb, in_=bo[:, :])

            # weights as lhsT chunks: [128 (d_in in chunk c), c, d_out]
            def load_w(name, dram, n_out):
                t = wts.tile([128, 4, n_out], F32R, tag=name)
                for c in range(4):
                    nc.sync.dma_start(out=t[:, c, :], in_=dram[c * 128:(c + 1) * 128, :])
                return t

            wq_sb = load_w("wq", wq, DIM)
            wk_sb = load_w("wk", wk, DIM)
            wv_sb = load_w("wv", wv, DIM)
            wgq_sb = load_w("wgq", wgq, RANK)
            wgk_sb = load_w("wgk", wgk, RANK)

            # persistent activations
            qpT = kvpool.tile([128, 4, Q], F32R, tag="qpT")     # q_p^T chunks
            gq_sb = kvpool.tile([32, Q], F32R, tag="gq")        # gate_q^T

            # ---- LN + transpose of one 128-row tile into xt_big[:, :, j*128:] ----
            def ln_transpose(src_dram, row0, nrows, xt_big, jcol):
                x = xin.tile([128, DIM], F32, tag="x")
                nc.sync.dma_start(out=x[:nrows, :], in_=src_dram[row0:row0 + nrows, :])
                st6 = stats.tile([128, 6], F32, tag="st6")
                nc.vector.bn_stats(out=st6[:nrows], in_=x[:nrows, :])
                mv = stats.tile([128, 2], F32, tag="mv")
                nc.vector.bn_aggr(out=mv[:nrows], in_=st6[:nrows])
                sd = stats.tile([128, 1], F32, tag="sd")
                nc.scalar.activation(out=sd[:nrows], in_=mv[:nrows, 1:2],
                                     func=FT.Sqrt, bias=eps[:nrows], scale=1.0)
                rstd = stats.tile([128, 1], F32, tag="rstd")
                nc.vector.reciprocal(out=rstd[:nrows], in_=sd[:nrows])
                nmr = stats.tile([128, 1], F32, tag="nmr")
                nc.vector.scalar_tensor_tensor(
                    out=nmr[:nrows], in0=mv[:nrows, 0:1], scalar=-1.0,
                    in1=rstd[:nrows], op0=ALU.mult, op1=ALU.mult)
                xn = xin.tile([128, DIM], F32, tag="xn")
                nc.vector.tensor_scalar_mul(xn[:nrows], x[:nrows, :], rstd[:nrows])
                nc.vector.tensor_scalar_add(xn[:nrows], xn[:nrows], nmr[:nrows])
                tp = ps_s.tile([128, 4, 128], F32, tag="ps")
                for c in range(4):
                    nc.tensor.transpose(tp[:, c, :nrows], xn[:nrows, c * 128:(c + 1) * 128], ident)
                nc.vector.tensor_copy(xt_big[:, :, jcol * 128:jcol * 128 + nrows], tp[:, :, :nrows])

            # ---------------- phase A: queries ----------------
            for sup in range(2):  # 512 q rows each
                qt_big = xtp.tile([128, 4, 512], F32R, tag="xt_big")
                for j in range(4):
                    ln_transpose(q, sup * 512 + j * 128, 128, qt_big, j)
                # q_p^T chunks for these 512 q columns
                for m in range(4):
                    pp = ps_s.tile([128, 512], F32, tag="ps")
                    for c in range(4):
                        nc.tensor.matmul(pp, r32(wq_sb[:, c, m * 128:(m + 1) * 128]),
                                         r32(qt_big[:, c, :]), start=(c == 0), stop=(c == 3))
                    nc.scalar.activation(out=qpT[:, m, sup * 512:(sup + 1) * 512], in_=pp,
                                         func=FT.Identity, bias=rqt_sb[:, m:m + 1], scale=1.0)
                # gate_q^T = WgqS^T @ q_p^T (contraction over q_p feature dim)
                gp = ps_s.tile([32, 512], F32, tag="ps_g", bufs=1)
                for c in range(4):
                    nc.tensor.matmul(gp, r32(wgq_sb[:, c, :]),
                                     qpT[:, c, sup * 512:(sup + 1) * 512],
                                     start=(c == 0), stop=(c == 3))
                nc.vector.tensor_copy(gq_sb[:, sup * 512:(sup + 1) * 512], gp)

            # wo shares wq's slot; loaded after last wq use (phase A done)
            wo_sb = load_w("wq", wo, DIM)

            # ---------------- kv phases ----------------
            for kv_start, n_kv, q_tiles in PHASES:
                kT = kvpool.tile([128, 4, KVW], F32R, tag="kT")
                vb = kvpool.tile([128, 17, DIM], F32R, tag="vb")
                gk_sb = kvpool.tile([32, KVW], F32R, tag="gk")

                for sup in range(5):  # supertiles of 4,4,4,4,1 kv tiles
                    j0 = sup * 4
                    nt = min(4, n_kv - j0)
                    ncols = nt * 128
                    st_big = xtp.tile([128, 4, 512], F32R, tag="xt_big")
                    for j in range(nt):
                        ln_transpose(s, kv_start + (j0 + j) * 128, 128, st_big, j)
                    # k_p^T chunks
                    for m in range(4):
                        pp = ps_s.tile([128, 512], F32, tag="ps")
                        for c in range(4):
                            nc.tensor.matmul(pp[:, :ncols], r32(wk_sb[:, c, m * 128:(m + 1) * 128]),
                                             r32(st_big[:, c, :ncols]), start=(c == 0), stop=(c == 3))
                        nc.scalar.activation(out=kT[:, m, j0 * 128:j0 * 128 + ncols], in_=pp[:, :ncols],
                                             func=FT.Identity, bias=rkt_sb[:, m:m + 1], scale=1.0)
                    # v_p natural rows
                    for j in range(nt):
                        pv = ps_s.tile([128, 512], F32, tag="ps")
                        for c in range(4):
                            nc.tensor.matmul(pv, r32(st_big[:, c, j * 128:(j + 1) * 128]),
                                             r32(wv_sb[:, c, :]), start=(c == 0), stop=False)
                        nc.tensor.matmul(pv, r32(ones1), r32(rv_sb), start=False, stop=True)
                        nc.scalar.copy(vb[:, j0 + j, :], pv)
                    # gate_k^T = Wgk^T @ k_p^T
                    gp = ps_s.tile([32, 512], F32, tag="ps_g", bufs=1)
                    for c in range(4):
                        nc.tensor.matmul(gp[:, :ncols], r32(wgk_sb[:, c, :]),
                                         kT[:, c, j0 * 128:j0 * 128 + ncols],
                                         start=(c == 0), stop=(c == 3))
                    nc.vector.tensor_copy(gk_sb[:, j0 * 128:j0 * 128 + ncols], gp[:, :ncols])

                if debug and kv_start == 0:
                    nc.sync.dma_start(out=d_kT[:, :, :], in_=kT[:, :, :].bitcast(F32))
                    nc.sync.dma_start(out=d_vb[:, :, :], in_=vb[:, :, :].bitcast(F32))
                    nc.sync.dma_start(out=d_gk[:, :], in_=gk_sb[:, :].bitcast(F32))
                    nc.sync.dma_start(out=d_qpT[:, :, :], in_=qpT[:, :, :].bitcast(F32))
                    nc.sync.dma_start(out=d_gq[:, :], in_=gq_sb[:, :].bitcast(F32))

                # ---------------- attention over this phase's q tiles ----------------
                for t in q_tiles:
                    if stage < 3:
                        ob0 = outp.tile([128, DIM], F32, tag="ob")
                        nc.vector.tensor_copy(ob0, vb[:, 0, :].bitcast(F32))
                        nc.sync.dma_start(out=out[t * 128:(t + 1) * 128, :], in_=ob0)
                        continue
                    w0 = WSTARTS[t]
                    rel = w0 - kv_start
                    qc = bass.ts(t, 128)
                    msk = attn.tile([128, WINW], F32, tag="msk")
                    nc.sync.dma_start(out=msk, in_=bmask[t, :, :])
                    # gate logits -> gate bias
                    gl = ps_b.tile([128, WINW], F32, tag="ps_big")
                    for n0 in (0, 512):
                        nn_ = min(512, WINW - n0)
                        nc.tensor.matmul(gl[:, n0:n0 + nn_], r32(gq_sb[:, qc]),
                                         r32(gk_sb[:, rel + n0:rel + n0 + nn_]),
                                         start=True, stop=True)
                    if stage == 30:
                        obx = outp.tile([128, DIM], F32, tag="ob")
                        nc.vector.tensor_copy(obx, gl[:, :DIM])
                        nc.sync.dma_start(out=out[t * 128:(t + 1) * 128, :], in_=obx)
                        continue
                    sig = attn.tile([128, WINW], F32, tag="sig")
                    nc.scalar.activation(out=sig, in_=gl, func=FT.Sigmoid, bias=zero_c)
                    gb = attn.tile([128, WINW], F32, tag="gb")
                    nc.scalar.activation(out=gb, in_=sig, func=FT.Ln, bias=eps6, scale=1.0)
                    if stage == 31:
                        obx = outp.tile([128, DIM], F32, tag="ob")
                        nc.vector.tensor_copy(obx, gb[:, :DIM])
                        nc.sync.dma_start(out=out[t * 128:(t + 1) * 128, :], in_=obx)
                        continue
                    # scores
                    sc = ps_b.tile([128, WINW], F32, tag="ps_big")
                    for n0 in (0, 512):
                        nn_ = min(512, WINW - n0)
                        for c in range(4):
                            nc.tensor.matmul(sc[:, n0:n0 + nn_], r32(qpT[:, c, qc]),
                                             r32(kT[:, c, rel + n0:rel + n0 + nn_]),
                                             start=(c == 0), stop=(c == 3))
                    if stage == 32:
                        obx = outp.tile([128, DIM], F32, tag="ob")
                        nc.vector.tensor_copy(obx, sc[:, :DIM])
                        nc.sync.dma_start(out=out[t * 128:(t + 1) * 128, :], in_=obx)
                        continue
                    S = attn.tile([128, WINW], F32, tag="S")
                    nc.vector.scalar_tensor_tensor(out=S, in0=sc, scalar=1.0, in1=msk,
                                                   op0=ALU.mult, op1=ALU.add)
                    SG = attn.tile([128, WINW], F32, tag="sig")
                    nc.vector.tensor_add(SG, S, gb)
                    mx = stats.tile([128, 1], F32, tag="mx")
                    nc.vector.tensor_reduce(out=mx, in_=SG, axis=AX.X, op=ALU.max)
                    nmx = stats.tile([128, 1], F32, tag="nmx")
                    nc.vector.tensor_scalar_mul(nmx, mx, -1.0)
                    if debug and t == 0:
                        nc.sync.dma_start(out=d_S[:, :], in_=SG)
                    P = attn.tile([128, WINW], F32, tag="P")
                    nc.scalar.activation(out=P, in_=SG, func=FT.Exp, bias=nmx, scale=1.0)
                    rsum = stats.tile([128, 1], F32, tag="rsum")
                    nc.vector.tensor_reduce(out=rsum, in_=P, axis=AX.X, op=ALU.add)
                    rinv = stats.tile([128, 1], F32, tag="rinv")
                    nc.vector.reciprocal(out=rinv, in_=rsum)
                    if stage < 4 or stage == 33:
                        ob1 = outp.tile([128, DIM], F32, tag="ob")
                        nc.vector.tensor_copy(ob1, P[:, :DIM])
                        nc.sync.dma_start(out=out[t * 128:(t + 1) * 128, :], in_=ob1)
                        continue
                    # attn^T (unnormalized)
                    pt = ps_b.tile([128, WINW], F32, tag="ps_big")
                    for cc in range(6):
                        nc.tensor.transpose(pt[:, cc * 128:(cc + 1) * 128],
                                            P[:, cc * 128:(cc + 1) * 128], ident)
                    aT = attn.tile([128, 6, 128], F32R, tag="aT")
                    nc.vector.tensor_copy(aT, pt.rearrange("p (a b) -> p a b", a=6))
                    # attn @ V
                    av = ps_s.tile([128, 512], F32, tag="ps")
                    for cc in range(6):
                        nc.tensor.matmul(av, r32(aT[:, cc, :]), r32(vb[:, rel // 128 + cc, :]),
                                         start=(cc == 0), stop=(cc == 5))
                    oa = outp.tile([128, DIM], F32, tag="oa")
                    nc.vector.tensor_scalar_mul(oa, av, rinv)  # normalize rows
                    if debug and t == 0:
                        nc.sync.dma_start(out=d_P[:, :], in_=P)
                        nc.sync.dma_start(out=d_oa[:, :], in_=oa)
                    if stage < 5:
                        nc.sync.dma_start(out=out[t * 128:(t + 1) * 128, :], in_=oa)
                        continue
                    # out = oa @ Wo + bo
                    ot = ps_s.tile([128, 4, 128], F32, tag="ps")
                    for c in range(4):
                        nc.tensor.transpose(ot[:, c, :], oa[:, c * 128:(c + 1) * 128], ident)
                    oaT = outp.tile([128, 4, 128], F32R, tag="oaT")
                    nc.vector.tensor_copy(oaT, ot)
                    fin = ps_s.tile([128, 512], F32, tag="ps")
                    for c in range(4):
                        nc.tensor.matmul(fin, r32(oaT[:, c, :]), r32(wo_sb[:, c, :]),
                                         start=(c == 0), stop=False)
                    nc.tensor.matmul(fin, r32(ones1), r32(bo_sb), start=False, stop=True)
                    ob = outp.tile([128, DIM], F32, tag="ob")
                    nc.vector.tensor_copy(ob, fin)
                    nc.sync.dma_start(out=out[t * 128:(t + 1) * 128, :], in_=ob)

    if not nc.is_finalized():
        nc.finalize()
    return nc


_NC_CACHE = None


def _get_nc():
    global _NC_CACHE
    if _NC_CACHE is None:
        _NC_CACHE = build_bass()
    return _NC_CACHE


def _host_fold(inputs):
    f32 = np.float32
    scale = f32(DIM ** -0.5)
    ctx0 = np.asarray(inputs["ctx0"], f32)
    ctx1 = np.asarray(inputs["ctx1"], f32)
    pre = ctx0 @ inputs["Wc0"] + inputs["bc0"] + ctx1 @ inputs["Wc1"] + inputs["bc1"]
    pre = np.asarray(pre, f32)
    h = pre / (1.0 + np.exp(-pre))
    gb = np.asarray(h @ inputs["Wf"] + inputs["bf"], f32)
    gamma, beta = gb[:, :DIM], gb[:, DIM:]

    qn_g = np.asarray(inputs["qn_g"], f32)
    qn_b = np.asarray(inputs["qn_b"], f32)
    kvn_g = np.asarray(inputs["kvn_g"], f32)
    kvn_b = np.asarray(inputs["kvn_b"], f32)
    Wq, bq = np.asarray(inputs["Wq"], f32), np.asarray(inputs["bq"], f32)
    Wk, bk = np.asarray(inputs["Wk"], f32), np.asarray(inputs["bk"], f32)
    Wv, bv = np.asarray(inputs["Wv"], f32), np.asarray(inputs["bv"], f32)
    mask = np.asarray(inputs["mask"], f32)

    WkS = np.ascontiguousarray((Wk * kvn_g[:, None]).astype(f32))
    r_k = (kvn_b @ Wk + bk).astype(f32)
    WvS = np.ascontiguousarray((Wv * kvn_g[:, None]).astype(f32))
    r_v = (kvn_b @ Wv + bv).astype(f32)
    WgqS = np.ascontiguousarray((inputs["Wgq"] / scale / np.sqrt(RANK)).astype(f32))
    Wgk = np.ascontiguousarray(np.asarray(inputs["Wgk"], f32))
    Wo = np.ascontiguousarray(np.asarray(inputs["Wo"], f32))
    bo = np.asarray(inputs["bo"], f32)

    bmask = np.stack([mask[t * 128:(t + 1) * 128, w:w + WINW]
                      for t, w in enumerate(WSTARTS)]).astype(f32)
    bmask = np.ascontiguousarray(np.maximum(bmask, -1e30))  # avoid -inf on device

    query = np.asarray(inputs["query"], f32).reshape(B, Q, DIM)
    source = np.asarray(inputs["source"], f32).reshape(B, K, DIM)

    in_maps = []
    for b in range(B):
        sg = (qn_g * (1.0 + gamma[b])).astype(f32)
        WqS = np.ascontiguousarray((Wq * sg[:, None] * scale).astype(f32))
        r_q = (((qn_b * (1.0 + gamma[b]) + beta[b]) @ Wq + bq) * scale).astype(f32)
        in_maps.append({
            "q": np.ascontiguousarray(query[b]),
            "s": np.ascontiguousarray(source[b]),
            "wq": WqS, "wk": WkS, "wv": WvS, "wo": Wo,
            "wgq": WgqS, "wgk": Wgk,
            "rqt": np.ascontiguousarray(r_q.reshape(4, 128).T),
            "rkt": np.ascontiguousarray(r_k.reshape(4, 128).T),
            "rv": r_v.reshape(1, DIM),
            "bo": bo.reshape(1, DIM),
            "bmask": bmask,
        })
    return in_maps


def kernel(**inputs):
    nc = _get_nc()
    in_maps = _host_fold(inputs)
    res = run_bass_kernel_spmd(nc, in_maps, core_ids=list(range(B)))
    out = np.stack([res.results[b]["out"] for b in range(B)])
    return out.reshape(B, QS, QT, DIM).astype(np.float32)


if __name__ == "__main__":
    build_bass()
    print("bass build OK")



# revision 8
# speedup vs baseline: 1.0104x; 1.0104x over previous
"""Trainium2 Bass kernel for nn_LocalDenseCrossReadout (v2, bf16).

Strategy:
- Data-parallel over batch: 8 batches -> 8 NeuronCores, one batch per core.
- Host-side (numpy, tiny): FiLM conditioning folded into per-batch q
  projection weights; LN affine + score scale folded; gate projections
  fused into the q/k projection weight matrices (544-wide outputs);
  v-projection bias folded into the output bias (softmax rows sum to 1);
  band mask sliced per q tile; all matmul operands pre-cast to bf16.
- Device kernel per core (single phase, everything resident in SBUF):
  LN (bn_stats/aggr + fused scale-bias apply on ScalarE) -> XBAR DMA
  transpose -> bf16 projections (k/gate_k and q/gate_q share one
  stationary weight load pattern; v in natural layout), then banded
  attention per 128-row q tile: gate logits -> tanh (sigmoid via
  0.5+0.5*tanh(x/2), same act table as exp), scores + mask added via
  identity matmul into PSUM, exp, P=(1+t)*e with fused row-sum,
  P^T/oa^T via XBAR DMA transpose, attn@V, output projection.
"""

import sys

sys.path.insert(0, "/opt/trn_rl_repo")

import numpy as np

import concourse.bass as bass
import concourse.tile as tile
from concourse import bacc
from concourse import mybir
from concourse.bass_utils import run_bass_kernel_spmd
from concourse.masks import make_identity

DIM, QS, QT, KS, KT, WIN, B, RANK = 512, 64, 16, 256, 16, 4, 8, 32
Q = QS * QT  # 1024
K = KS * KT  # 4096
WINW = 768  # aligned kv window per 128-row q tile
NQT = Q // 128  # 8 q tiles
NKV = K // 128  # 32 kv tiles
F32 = mybir.dt.float32
BF16 = mybir.dt.bfloat16
FT = mybir.ActivationFunctionType
ALU = mybir.AluOpType
AX = mybir.AxisListType

# kv window start (aligned to 128) per q tile
WSTARTS = [0, 384, 896, 1408, 1920, 2432, 2944, 3328]


def build_bass():
    nc = bacc.Bacc("TRN2", target_bir_lowering=False)
    q = nc.dram_tensor("q", [Q, DIM], BF16, kind="ExternalInput")
    s = nc.dram_tensor("s", [K, DIM], BF16, kind="ExternalInput")
    wq = nc.dram_tensor("wq", [DIM, DIM + RANK], BF16, kind="ExternalInput")
    wk = nc.dram_tensor("wk", [DIM, DIM + RANK], BF16, kind="ExternalInput")
    wv = nc.dram_tensor("wv", [DIM, DIM], BF16, kind="ExternalInput")
    wo = nc.dram_tensor("wo", [DIM, DIM], BF16, kind="ExternalInput")
    rqt = nc.dram_tensor("rqt", [128, 5], F32, kind="ExternalInput")
    rkt = nc.dram_tensor("rkt", [128, 5], F32, kind="ExternalInput")
    bo2 = nc.dram_tensor("bo2", [1, DIM], BF16, kind="ExternalInput")
    bmask = nc.dram_tensor("bmask", [NQT, 128, WINW], BF16, kind="ExternalInput")
    out = nc.dram_tensor("out", [Q, DIM], F32, kind="ExternalOutput")

    with tile.TileContext(nc) as tc:
        with (
            tc.tile_pool(name="consts", bufs=1) as consts,
            tc.tile_pool(name="wts", bufs=1) as wts,
            tc.tile_pool(name="big", bufs=1) as big,
            tc.tile_pool(name="xin", bufs=4) as xin,
            tc.tile_pool(name="xnp", bufs=4) as xnp,
            tc.tile_pool(name="stats", bufs=6) as stats,
            tc.tile_pool(name="attn", bufs=2) as attn,
            tc.tile_pool(name="msks", bufs=1) as msks,
            tc.tile_pool(name="ps_s", bufs=3, space="PSUM") as ps_s,
            tc.tile_pool(name="ps_b", bufs=2, space="PSUM") as ps_b,
        ):
            # ---------------- constants ----------------
            eps = consts.tile([128, 1], F32)
            nc.vector.memset(eps, 1e-5)
            identb = consts.tile([128, 128], BF16)
            make_identity(nc, identb)
            ones1 = consts.tile([1, 128], BF16)
            nc.vector.memset(ones1, 1.0)
            rqt_sb = consts.tile([128, 5], F32)
            nc.sync.dma_start(out=rqt_sb, in_=rqt[:, :])
            rkt_sb = consts.tile([128, 5], F32)
            nc.sync.dma_start(out=rkt_sb, in_=rkt[:, :])
            bo2_sb = consts.tile([1, DIM], BF16)
            nc.sync.dma_start(out=bo2_sb, in_=bo2[:, :])

            # weights as lhsT chunks: [128 (d_in in chunk c), c, d_out]
            def load_w(name, dram, n_out):
                t = wts.tile([128, 4, n_out], BF16, tag=name)
                for c in range(4):
                    nc.sync.dma_start(out=t[:, c, :], in_=dram[c * 128:(c + 1) * 128, :])
                return t

            wq_sb = load_w("wq", wq, DIM + RANK)
            wk_sb = load_w("wk", wk, DIM + RANK)
            wv_sb = load_w("wv", wv, DIM)
            wo_sb = load_w("wo", wo, DIM)

            # persistent activations (all bf16)
            qt_big = big.tile([128, 4, Q], BF16, tag="qt_big")    # xn_q^T
            st_big = big.tile([128, 4, K], BF16, tag="st_big")    # xn_s^T
            qpT = big.tile([128, 4, Q], BF16, tag="qpT")          # q_p^T
            gq = big.tile([32, Q], BF16, tag="gq")                # gate_q^T
            kT = big.tile([128, 4, K], BF16, tag="kT")            # k_p^T
            gk = big.tile([32, K], BF16, tag="gk")                # gate_k^T
            vb = big.tile([128, NKV, DIM], BF16, tag="vb")        # v_p rows

            # ---- LN one 128-row tile -> transposed into dst[:, :, col0:col0+128]
            def ln_tile(src_dram, row0, dst_big, col0):
                x = xin.tile([128, DIM], BF16, tag="x")
                nc.sync.dma_start(out=x, in_=src_dram[row0:row0 + 128, :])
                st6 = stats.tile([128, 6], F32, tag="st6")
                nc.vector.bn_stats(out=st6, in_=x)
                mv = stats.tile([128, 2], F32, tag="mv")
                nc.vector.bn_aggr(out=mv, in_=st6)
                sd = stats.tile([128, 1], F32, tag="sd")
                nc.scalar.activation(out=sd, in_=mv[:, 1:2],
                                     func=FT.Sqrt, bias=eps, scale=1.0)
                rstd = stats.tile([128, 1], F32, tag="rstd")
                nc.vector.reciprocal(out=rstd, in_=sd)
                nmr = stats.tile([128, 1], F32, tag="nmr")
                nc.vector.scalar_tensor_tensor(
                    out=nmr, in0=mv[:, 0:1], scalar=-1.0,
                    in1=rstd, op0=ALU.mult, op1=ALU.mult)
                xn = xnp.tile([128, DIM], BF16, tag="xn")
                nc.gpsimd.tensor_scalar(out=xn, in0=x, scalar1=rstd, scalar2=nmr,
                                        op0=ALU.mult, op1=ALU.add)
                # XBAR transpose: dst[dlow, c, col0+i] = xn[i, c*128+dlow]
                nc.scalar.dma_start_transpose(dst_big[:, :, col0:col0 + 128], xn)

            # ---- projections for one bank of 512 rows (cols of the T layout)
            def proj_bank(w_sb, src_big, col0, dstT, dstG, r_sb):
                for m in range(5):
                    mw = 128 if m < 4 else RANK
                    mo = m * 128
                    pp = ps_s.tile([128, DIM], F32, tag="ps")
                    for c in range(4):
                        nc.tensor.matmul(pp[:mw, :], w_sb[:, c, mo:mo + mw],
                                         src_big[:, c, col0:col0 + 512],
                                         start=(c == 0), stop=(c == 3))
                    if m < 4:
                        nc.scalar.activation(out=dstT[:, m, col0:col0 + 512],
                                             in_=pp, func=FT.Identity,
                                             bias=r_sb[:, m:m + 1], scale=1.0)
                    else:
                        nc.scalar.activation(out=dstG[:, col0:col0 + 512],
                                             in_=pp[:RANK, :], func=FT.Identity,
                                             bias=r_sb[:RANK, 4:5], scale=1.0)

            # ---------------- queries ----------------
            for qb in range(2):
                for j in range(4):
                    ln_tile(q, qb * 512 + j * 128, qt_big, qb * 512 + j * 128)
                proj_bank(wq_sb, qt_big, qb * 512, qpT, gq, rqt_sb)

            # ---------------- keys/values ----------------
            for kb in range(8):
                for j in range(4):
                    ln_tile(s, kb * 512 + j * 128, st_big, kb * 512 + j * 128)
                proj_bank(wk_sb, st_big, kb * 512, kT, gk, rkt_sb)
                for j in range(4):
                    jj = kb * 4 + j
                    pv = ps_s.tile([128, DIM], F32, tag="ps")
                    for c in range(4):
                        nc.tensor.matmul(pv, st_big[:, c, jj * 128:(jj + 1) * 128],
                                         wv_sb[:, c, :], start=(c == 0), stop=(c == 3))
                    nc.scalar.copy(vb[:, jj, :], pv)

            # ---------------- attention ----------------
            # prefetch all band masks
            msk_t = []
            for t in range(NQT):
                m_ = msks.tile([128, WINW], BF16, tag=f"msk{t}")
                nc.sync.dma_start(out=m_, in_=bmask[t, :, :])
                msk_t.append(m_)

            for t in range(NQT):
                w0 = WSTARTS[t]
                qc = bass.ts(t, 128)
                # gate logits
                gl = ps_b.tile([128, WINW], F32, tag="glsc")
                for n0, nn_ in ((0, 512), (512, 256)):
                    nc.tensor.matmul(gl[:, n0:n0 + nn_], gq[:, qc],
                                     gk[:, w0 + n0:w0 + n0 + nn_],
                                     start=True, stop=True)
                # 2*sigmoid(gl) = 1 + tanh(gl/2); factor 2 cancels in softmax
                tq = attn.tile([128, WINW], BF16, tag="tq")
                nc.scalar.activation(out=tq, in_=gl, func=FT.Tanh,
                                     bias=0.0, scale=0.5)
                # scores + mask (mask added via identity matmul into PSUM)
                sc = ps_b.tile([128, WINW], F32, tag="glsc")
                for n0, nn_ in ((0, 512), (512, 256)):
                    for c in range(4):
                        nc.tensor.matmul(sc[:, n0:n0 + nn_], qpT[:, c, qc],
                                         kT[:, c, w0 + n0:w0 + n0 + nn_],
                                         start=(c == 0), stop=False)
                    nc.tensor.matmul(sc[:, n0:n0 + nn_], identb,
                                     msk_t[t][:, n0:n0 + nn_],
                                     start=False, stop=True)
                e = attn.tile([128, WINW], BF16, tag="e")
                nc.scalar.activation(out=e, in_=sc, func=FT.Exp, bias=0.0)
                # P = (1 + tanh) * e, with fused row-sum
                P = attn.tile([128, WINW], BF16, tag="P")
                rsum = stats.tile([128, 1], F32, tag="rsum")
                nc.vector.scalar_tensor_tensor(
                    out=P, in0=tq, scalar=1.0, in1=e,
                    op0=ALU.add, op1=ALU.mult, accum_out=rsum)
                rinv = stats.tile([128, 1], F32, tag="rinv")
                nc.vector.reciprocal(out=rinv, in_=rsum)
                # P^T via XBAR transpose
                aT = attn.tile([128, 6, 128], BF16, tag="aT")
                nc.sync.dma_start_transpose(aT, P)
                # attn @ V
                av = ps_s.tile([128, DIM], F32, tag="ps")
                for cc in range(6):
                    nc.tensor.matmul(av, aT[:, cc, :], vb[:, w0 // 128 + cc, :],
                                     start=(cc == 0), stop=(cc == 5))
                oa = attn.tile([128, DIM], BF16, tag="oa")
                nc.vector.tensor_scalar_mul(oa, av, rinv)
                oaT = attn.tile([128, 4, 128], BF16, tag="oaT")
                nc.sync.dma_start_transpose(oaT, oa)
                # out = oa @ Wo + bo2
                fin = ps_s.tile([128, DIM], F32, tag="ps")
                for c in range(4):
                    nc.tensor.matmul(fin, oaT[:, c, :], wo_sb[:, c, :],
                                     start=(c == 0), stop=False)
                nc.tensor.matmul(fin, ones1, bo2_sb, start=False, stop=True)
                ob = attn.tile([128, DIM], F32, tag="ob")
                nc.scalar.copy(ob, fin)
                nc.sync.dma_start(out=out[qc, :], in_=ob)

    if not nc.is_finalized():
        nc.finalize()
    return nc


_NC_CACHE = None


def _get_nc():
    global _NC_CACHE
    if _NC_CACHE is None:
        _NC_CACHE = build_bass()
    return _NC_CACHE


def _host_fold(inputs):
    f32 = np.float32
    bf16 = mybir.dt.np(BF16)
    scale = f32(DIM ** -0.5)
    sqr = f32(np.sqrt(RANK))
    ctx0 = np.asarray(inputs["ctx0"], f32)
    ctx1 = np.asarray(inputs["ctx1"], f32)
    pre = ctx0 @ inputs["Wc0"] + inputs["bc0"] + ctx1 @ inputs["Wc1"] + inputs["bc1"]
    pre = np.asarray(pre, f32)
    h = pre / (1.0 + np.exp(-pre))
    gbv = np.asarray(h @ inputs["Wf"] + inputs["bf"], f32)
    gamma, beta = gbv[:, :DIM], gbv[:, DIM:]

    qn_g = np.asarray(inputs["qn_g"], f32)
    qn_b = np.asarray(inputs["qn_b"], f32)
    kvn_g = np.asarray(inputs["kvn_g"], f32)
    kvn_b = np.asarray(inputs["kvn_b"], f32)
    Wq, bq = np.asarray(inputs["Wq"], f32), np.asarray(inputs["bq"], f32)
    Wk, bk = np.asarray(inputs["Wk"], f32), np.asarray(inputs["bk"], f32)
    Wv, bv = np.asarray(inputs["Wv"], f32), np.asarray(inputs["bv"], f32)
    Wo, bo = np.asarray(inputs["Wo"], f32), np.asarray(inputs["bo"], f32)
    Wgq = np.asarray(inputs["Wgq"], f32)
    Wgk = np.asarray(inputs["Wgk"], f32)
    mask = np.asarray(inputs["mask"], f32)

    # k path (batch-independent): LN affine folded; gate_k fused as extra cols
    WkS = Wk * kvn_g[:, None]
    rk = (kvn_b @ Wk + bk).astype(f32)
    wk_ext = np.concatenate([WkS, WkS @ Wgk], axis=1).astype(bf16)
    rkt = np.zeros((128, 5), f32)
    rkt[:, :4] = rk.reshape(4, 128).T
    rkt[:RANK, 4] = rk @ Wgk
    # v path: bias folded into output bias (attn rows sum to 1)
    WvS = (Wv * kvn_g[:, None]).astype(bf16)
    rv = (kvn_b @ Wv + bv).astype(f32)
    bo2 = (rv @ Wo + bo).reshape(1, DIM).astype(bf16)
    Wo_b = np.ascontiguousarray(Wo).astype(bf16)

    bmask = np.stack([mask[t * 128:(t + 1) * 128, w:w + WINW]
                      for t, w in enumerate(WSTARTS)])
    bmask = np.maximum(bmask, -1e30).astype(bf16)

    query = np.asarray(inputs["query"], f32).reshape(B, Q, DIM)
    source = np.asarray(inputs["source"], f32).reshape(B, K, DIM)

    in_maps = []
    for b in range(B):
        sg = qn_g * (1.0 + gamma[b])
        off = qn_b * (1.0 + gamma[b]) + beta[b]
        Wq_f = Wq * sg[:, None]
        rq_raw = (off @ Wq + bq).astype(f32)
        wq_ext = np.concatenate([Wq_f * scale, (Wq_f @ Wgq) / sqr], axis=1)
        rqt_b = np.zeros((128, 5), f32)
        rqt_b[:, :4] = (rq_raw * scale).reshape(4, 128).T
        rqt_b[:RANK, 4] = rq_raw @ Wgq / sqr
        in_maps.append({
            "q": query[b].astype(bf16),
            "s": source[b].astype(bf16),
            "wq": wq_ext.astype(bf16),
            "wk": wk_ext,
            "wv": WvS,
            "wo": Wo_b,
            "rqt": rqt_b,
            "rkt": rkt,
            "bo2": bo2,
            "bmask": bmask,
        })
    return in_maps


def kernel(**inputs):
    nc = _get_nc()
    in_maps = _host_fold(inputs)
    res = run_bass_kernel_spmd(nc, in_maps, core_ids=list(range(B)))
    out = np.stack([res.results[b]["out"] for b in range(B)])
    return out.reshape(B, QS, QT, DIM).astype(np.float32)


if __name__ == "__main__":
    build_bass()
    print("bass build OK")
